# revision 12
# baseline (speedup 1.0000x reference)
"""Causal self-attention (GQA, rope, qk-rmsnorm) Trainium2 kernel, 8 NeuronCores.

Sharding: core = (b, g), b = core // 4 (batch), g = core % 4.
Row-sharded Q/attention/output (query row-chunks {g, 4+g, 8+g, 12+g} per core)
with HEAD-SHARDED K/V projection: each core computes only kv-head g's K and V
over all T tokens, then the per-batch 4-core groups AllGather K/V.

v2 scheduling changes vs the 408us baseline:
- attention exp runs on the ACT engine in 5 packed multi-bank psum tiles per
  head (instead of 16 narrow ones) cutting per-instruction bubbles ~3x
- softmax-denominator presum adds moved from the (busy) vector engine to the
  (idle) gpsimd engine
- x slabs double-buffered; initial DMAs ordered so the first matmul starts
  ~4us in (plus PE warmup transposes to ramp the p-state clock early)
- wq/xoTd prefetched during the K/V phase; output stored bf16 (host casts)

Slot c (c = 0..3) covers query chunk 4c+g with keys [0, 512*(c+1)); causal
masking inside the last 512 keys comes from a host-provided 0/1 multiplicative
mask applied to exp(S).
"""

import sys

if "/opt/trn_rl_repo" not in sys.path:
    sys.path.insert(0, "/opt/trn_rl_repo")

import ml_dtypes
import numpy as np

BF = ml_dtypes.bfloat16

B, T, C = 2, 2048, 2048
NH, NKV = 16, 4
HD = C // NH  # 128
P = 128
NT = T // P            # 16 token tiles per batch
NCT = C // P           # 16 contraction tiles
QROWS = 512            # own query rows per core
NQT = QROWS // P       # 4 own token tiles
EPS = float(np.finfo(np.float32).eps)

# attention score-tile packing: 5 psum tiles of [128, 1024] per head.
# each entry: (tile_idx, col_offset, kt, n) where n = valid query cols.
# tile A: kt0,kt1 (n=512); B: kt2,kt3; C: kt4(384),kt12(128),kt5,kt13;
# D: kt6,kt14,kt7,kt15; E: kt8..kt11 (n=256).
PACK = [
    (0, [(0, 0, 512), (512, 1, 512)]),
    (1, [(0, 2, 512), (512, 3, 512)]),
    (2, [(0, 4, 384), (384, 12, 128), (512, 5, 384), (896, 13, 128)]),
    (3, [(0, 6, 384), (384, 14, 128), (512, 7, 384), (896, 15, 128)]),
    (4, [(0, 8, 256), (256, 9, 256), (512, 10, 256), (768, 11, 256)]),
]
# presum groups for the denominator: 4 kt of equal width each; the acc is the
# group's first slice (in issue order).
GROUPS = {0: [0, 1, 2, 3], 1: [4, 5, 6, 7], 2: [8, 9, 10, 11], 3: [12, 13, 14, 15]}

_CACHE = {}


def _chunks(g):
    return [g, 4 + g, 8 + g, 12 + g]


def _rows(g):
    return np.concatenate([np.arange(ch * P, (ch + 1) * P) for ch in _chunks(g)])


def _qmask_t(g):
    """Multiplicative 0/1 mask, transposed layout: (slot c, sub s, k i, q j).

    For slot c the prob tile is P^T[k, q] with k in [0, 512*(c+1)) and q the
    128 rows of chunk 4c+g. Only keys in the last 512 of the slot can be
    invalid; mask[c, s, i, j] = 1 if key (512*c + s*128 + i) <= query
    (128*(4c+g) + j) else 0. Applied to exp(S) by elementwise multiply.
    """
    m = np.zeros((4, 4, P, P), np.float32)
    for c in range(4):
        k0 = 512 * c
        r0 = (4 * c + g) * P
        k = k0 + np.arange(512)[:, None]          # (512, 1)
        q = r0 + np.arange(P)[None, :]            # (1, 128)
        m[c] = np.where(k <= q, 1.0, 0.0).reshape(4, P, P)
    return m.astype(BF)


def _build():
    import concourse.bacc as bacc
    import concourse.bass as bass
    import concourse.mybir as mybir
    import concourse.tile as tile
    from concourse.masks import make_identity

    f32 = mybir.dt.float32
    bf16 = mybir.dt.bfloat16
    AF = mybir.ActivationFunctionType
    OP = mybir.AluOpType
    AX = mybir.AxisListType

    nc = bacc.Bacc("TRN2", target_bir_lowering=False, debug=False, num_devices=8)

    xfT = nc.dram_tensor("xfT", [C, T], bf16, kind="ExternalInput").ap()
    xoTd = nc.dram_tensor("xoTd", [C, QROWS], bf16, kind="ExternalInput").ap()
    cosf = nc.dram_tensor("cosf", [T, HD // 2], f32, kind="ExternalInput").ap()
    sinf = nc.dram_tensor("sinf", [T, HD // 2], f32, kind="ExternalInput").ap()
    coso = nc.dram_tensor("coso", [QROWS, HD // 2], f32, kind="ExternalInput").ap()
    sino = nc.dram_tensor("sino", [QROWS, HD // 2], f32, kind="ExternalInput").ap()
    wq = nc.dram_tensor("wq", [C, C], bf16, kind="ExternalInput").ap()
    wkvh = nc.dram_tensor("wkvh", [C, 2 * HD], bf16, kind="ExternalInput").ap()
    wo = nc.dram_tensor("wo", [C, C], bf16, kind="ExternalInput").ap()
    qm = nc.dram_tensor("qm", [4, 4, P, P], bf16, kind="ExternalInput").ap()
    yo = nc.dram_tensor("yo", [QROWS, C], bf16, kind="ExternalOutput").ap()

    def bcast4(ap2d):
        # [128, 64] -> [128, 4, 64] with middle step 0 (replicate across heads)
        return bass.AP(
            tensor=ap2d.tensor,
            offset=ap2d.offset,
            ap=[ap2d.ap[0], [0, 4], ap2d.ap[1]],
        )

    with tile.TileContext(nc) as tc:
        with (
            tc.tile_pool(name="singles", bufs=1) as singles,
            tc.tile_pool(name="big", bufs=1) as bigpool,
            tc.tile_pool(name="xq", bufs=2) as xqpool,
            tc.tile_pool(name="slab", bufs=3) as slabpool,
            tc.tile_pool(name="cs", bufs=2) as cspool,
            tc.tile_pool(name="epi", bufs=1) as epipool,
            tc.tile_pool(name="qh", bufs=2) as qhpool,
            tc.tile_pool(name="pt", bufs=4) as ptpool,
            tc.tile_pool(name="smallf", bufs=2) as smallf,
            tc.tile_pool(name="stg", bufs=3) as stgpool,
            tc.tile_pool(name="outs", bufs=2) as outpool,
            tc.tile_pool(name="dram", bufs=1, space="DRAM") as drampool,
            tc.tile_pool(name="psS", bufs=2, space="PSUM") as psS,
            tc.tile_pool(name="psY", bufs=2, space="PSUM") as psY,
            tc.tile_pool(name="psD", bufs=2, space="PSUM") as psD,
        ):
            # ---- first: the loads the first matmuls depend on ----
            def load_slab(w_ap, col0, cols, name):
                """A [C, cols<=512] slice of a weight as [128, 16, cols] bf16.

                All weight slabs (wkv, wq, wo) share one 3-slot pool; the
                phases are disjoint so slots time-share SBUF."""
                wsl = slabpool.tile([P, NCT, 512], bf16, tag="slab", name=name)
                for gr in range(4):
                    nc.sync.dma_start(
                        out=wsl[:, 4 * gr:4 * gr + 4, 0:cols],
                        in_=w_ap[:, col0:col0 + cols].rearrange(
                            "(a p) n -> p a n", p=P
                        )[:, 4 * gr:4 * gr + 4, :],
                    )
                return wsl

            wslkv = load_slab(wkvh, 0, 256, "wkv")

            def load_xe(e):
                """One x eighth ([C, 256]) as 4 tiles of [P, 4, 256]
                (double-buffered)."""
                xh = [xqpool.tile([P, 4, 256], bf16, tag=f"xT{gr}",
                                  name=f"xfT{e}{gr}")
                      for gr in range(4)]
                for gr in range(4):
                    nc.sync.dma_start(
                        out=xh[gr],
                        in_=xfT[gr * 512:(gr + 1) * 512,
                                e * 256:(e + 1) * 256].rearrange(
                            "(a p) n -> p a n", p=P),
                    )
                return xh

            xh_next = load_xe(0)

            ident = singles.tile([P, P], bf16)
            make_identity(nc, ident)

            # PE warmup: dummy transposes ramp the tensor-engine p-state while
            # the first x/w DMAs stream in (~4us of low/mid-clock work).
            warm = psY.tile([P, P], bf16, tag="Y", name="warm")
            for _ in range(24):
                nc.tensor.transpose(warm, ident, ident)

            ones128 = singles.tile([P, P], bf16)
            nc.vector.memset(ones128, 1.0)
            eps_q = singles.tile([P, 1], f32)
            nc.vector.memset(eps_q, EPS)
            eps_k = singles.tile([P, 1], f32)
            nc.vector.memset(eps_k, HD * EPS)

            # persistent big SBUF tensors
            qT = bigpool.tile([P, NH, QROWS], bf16, tag="qT")      # [d, h, q]
            kT = bigpool.tile([P, NKV, T], bf16, tag="kT")         # [d, kvh, k]
            vA = bigpool.tile([P, NT, NKV, HD], bf16, tag="vA")    # [ktok, tt, kvh, d]
            yT = bigpool.tile([P, NCT, QROWS], bf16, tag="yT")     # [d, ct, q]
            qmask = singles.tile([P, 4, 4, P], bf16)               # [ki, c, sub, q]
            nc.sync.dma_start(out=qmask, in_=qm.rearrange("c s i j -> i c s j"))

            # DRAM bounce buffers for the K/V AllGather (two halves so the
            # first collective starts while the second half is computed)
            sendh = [drampool.tile([P, 1024 * (1 if i == 0 else 3)], bf16,
                                   tag=f"send{i}", name=f"sendh{i}")
                     for i in range(2)]
            recvh = [drampool.tile([4, P, 1024 * (1 if i == 0 else 3)], bf16,
                                   tag=f"recv{i}", name=f"recvh{i}")
                     for i in range(2)]

            # ---------------- helpers ----------------
            pending = []  # delayed PE transpose packs (2-deep pipeline)

            def drain_pending(keep=0):
                while len(pending) > keep:
                    pending.pop(0)()

            def rope_rms(v3, cosn, sinn, out_bf, eps_ap, sqrt_scale, nh):
                """v3: [128, nh, 128] psum f32 view. Writes normalized bf16
                rope output to out_bf [128, nh, 128]."""
                ro = epipool.tile([P, 4, HD], f32, tag="ro", name="ro")[:, 0:nh, :]
                cs = epipool.tile([P, 4, HD], f32, tag="cs", name="cs")[:, 0:nh, :]
                sn = epipool.tile([P, 4, HD], f32, tag="sn", name="sn")[:, 0:nh, :]
                nc.vector.tensor_tensor(cs[:, :, 0:64], v3[:, :, 0:64], cosn, op=OP.mult)
                nc.vector.tensor_tensor(cs[:, :, 64:128], v3[:, :, 64:128], cosn, op=OP.mult)
                nc.vector.tensor_tensor(sn[:, :, 0:64], v3[:, :, 0:64], sinn, op=OP.mult)
                nc.vector.tensor_tensor(sn[:, :, 64:128], v3[:, :, 64:128], sinn, op=OP.mult)
                nc.vector.tensor_tensor(ro[:, :, 0:64], cs[:, :, 0:64], sn[:, :, 64:128], op=OP.add)
                nc.vector.tensor_sub(ro[:, :, 64:128], cs[:, :, 64:128], sn[:, :, 0:64])
                ss = smallf.tile([P, 4], f32, tag="ss", name="ss")[:, 0:nh]
                sq = epipool.tile([P, 4, HD], f32, tag="cs", name="sq")[:, 0:nh, :]
                nc.vector.tensor_tensor(sq, ro, ro, op=OP.mult)
                nc.vector.reduce_sum(ss, sq, axis=AX.X)
                rms = smallf.tile([P, 4], f32, tag="rms", name="rms")[:, 0:nh]
                nc.scalar.activation(rms, ss, AF.Sqrt, bias=eps_ap, scale=sqrt_scale)
                rinv = smallf.tile([P, 4], f32, tag="rms", name="rinv")[:, 0:nh]
                nc.vector.reciprocal_approx_fast(rinv, rms)
                for hh in range(nh):
                    nc.vector.tensor_scalar_mul(
                        out_bf[:, hh, :], ro[:, hh, :], rinv[:, hh:hh + 1]
                    )

            def pack_transpose(src_bf, dst3, nh):
                """src_bf [128, nh, 128] bf16 -> nh PE transposes -> one copy
                to dst3 ([128, nh, 128] view)."""
                ptr = psY.tile([P, 512], bf16, tag="Y", name="ptrq")
                for hh in range(nh):
                    nc.tensor.transpose(
                        ptr[:, hh * P:(hh + 1) * P], src_bf[:, hh, :], ident
                    )
                nc.vector.tensor_copy(
                    dst3, ptr[:, 0:nh * P].rearrange("p (s n) -> p s n", s=nh)
                )

            def cos_tiles(cap, sap, t0, nh, name):
                cosn = cspool.tile([P, 4, 64], f32, tag="cs4", name=f"c{name}")[:, 0:nh, :]
                sinn = cspool.tile([P, 4, 64], f32, tag="sn4", name=f"s{name}")[:, 0:nh, :]
                if nh == 4:
                    nc.scalar.dma_start(out=cosn, in_=bcast4(cap[t0:t0 + P, :]))
                    nc.scalar.dma_start(out=sinn, in_=bcast4(sap[t0:t0 + P, :]))
                else:
                    nc.scalar.dma_start(
                        out=cosn,
                        in_=cap[t0:t0 + P, :].rearrange("p (h d) -> p h d", h=1))
                    nc.scalar.dma_start(
                        out=sinn,
                        in_=sap[t0:t0 + P, :].rearrange("p (h d) -> p h d", h=1))
                return cosn, sinn

            # ---------------- phase 0: local kv-head K/V projection --------
            def k_epilogue(kstage, quarter):
                # one 4-wide rope+rms+pack for the whole quarter
                cos4q = cspool.tile([P, 4, 64], f32, tag="cs4", name=f"cq{quarter}")
                sin4q = cspool.tile([P, 4, 64], f32, tag="sn4", name=f"sq{quarter}")
                nc.scalar.dma_start(
                    out=cos4q,
                    in_=cosf[quarter * 512:(quarter + 1) * 512, :].rearrange(
                        "(a p) d -> p a d", p=P))
                nc.scalar.dma_start(
                    out=sin4q,
                    in_=sinf[quarter * 512:(quarter + 1) * 512, :].rearrange(
                        "(a p) d -> p a d", p=P))
                khat4 = qhpool.tile([P, 4, HD], bf16, tag="khat", name=f"kh{quarter}")
                rope_rms(kstage, cos4q, sin4q, khat4, eps_k, 1.0, 4)
                kst4 = stgpool.tile([P, 4, HD], bf16, tag="kst", name=f"kst{quarter}")
                pack_transpose(khat4, kst4, 4)
                if quarter == 0:
                    nc.sync.dma_start(out=sendh[0][:, 0:512],
                                      in_=kst4.rearrange("p a d -> p (a d)"))
                else:
                    q1 = quarter - 1
                    nc.sync.dma_start(out=sendh[1][:, q1 * 512:(q1 + 1) * 512],
                                      in_=kst4.rearrange("p a d -> p (a d)"))

            def kv_allgather(i):
                nc.gpsimd.collective_compute(
                    "AllGather",
                    mybir.AluOpType.bypass,
                    replica_groups=[[0, 1, 2, 3], [4, 5, 6, 7]],
                    ins=[sendh[i].opt()],
                    outs=[recvh[i].opt()],
                )
                kw = 512 if i == 0 else 1536
                t0, t1 = (0, 512) if i == 0 else (512, 2048)
                nc.gpsimd.dma_start(
                    out=kT[:, :, t0:t1],
                    in_=recvh[i][:, :, 0:kw].rearrange("r p n -> p r n"),
                )
                for r in range(4):
                    nc.gpsimd.dma_start(
                        out=vA[:, t0 // P:t1 // P, r, :],
                        in_=recvh[i][r, :, kw:2 * kw].rearrange(
                            "p (tt d) -> p tt d", tt=(t1 - t0) // P),
                    )

            # xoT for the Q projection: loaded early, spread across phase 0
            xoT = [
                bigpool.tile([P, 4, QROWS], bf16, tag=f"xoT{gr}", name=f"xoT{gr}")
                for gr in range(4)
            ]

            def load_xo(tt):
                for gr in range(4):
                    nc.sync.dma_start(
                        out=xoT[gr][:, :, tt * P:(tt + 1) * P],
                        in_=xoTd[gr * 512:(gr + 1) * 512,
                                 tt * P:(tt + 1) * P].rearrange(
                            "(a p) n -> p a n", p=P),
                    )

            wsl_pre = {}
            for quarter in range(4):
                kstage = stgpool.tile([P, 4, HD], f32, tag="kstage",
                                      name=f"kstage{quarter}")
                vstage = stgpool.tile([P, 4, HD], bf16, tag="vstage",
                                      name=f"vstage{quarter}")
                for half in range(2):
                    e = 2 * quarter + half
                    xhT = xh_next
                    if e < 7:
                        xh_next = load_xe(e + 1)
                    # spread the phase-1 prefetches across phase-0 order
                    if e == 0:
                        load_xo(0)
                    elif e == 2:
                        load_xo(1)
                    elif e == 3:
                        load_xo(2)
                        wsl_pre[0] = load_slab(wq, 0, 512, "wq0")
                    elif e == 5:
                        load_xo(3)
                        wsl_pre[1] = load_slab(wq, 512, 512, "wq1")
                    for tl in range(2 * half, 2 * half + 2):
                        tl2 = tl - 2 * half
                        pskv = psS.tile([P, 256], f32, tag="S", name="pskv")
                        for ct in range(NCT):
                            nc.tensor.matmul(
                                pskv,
                                xhT[ct // 4][:, ct % 4, tl2 * P:(tl2 + 1) * P],
                                wslkv[:, ct, 0:256],
                                start=(ct == 0),
                                stop=(ct == NCT - 1),
                            )
                        nc.scalar.copy(kstage[:, tl, :], pskv[:, 0:HD])
                        nc.scalar.copy(vstage[:, tl, :], pskv[:, HD:2 * HD])
                vo0 = 512 if quarter == 0 else 1536 + (quarter - 1) * 512
                nc.sync.dma_start(
                    out=sendh[min(quarter, 1)][:, vo0:vo0 + 512],
                    in_=vstage.rearrange("p a d -> p (a d)"))
                if quarter == 0:
                    k_epilogue(kstage, 0)
                    kv_allgather(0)
                else:
                    drain_pending(1)
                    pending.append(
                        lambda kstage=kstage, quarter=quarter: k_epilogue(
                            kstage, quarter))
            drain_pending()
            kv_allgather(1)

            # ---------------- phase 1: Q projection ----------------
            for s in range(4):
                wsl = wsl_pre.pop(s, None)
                if wsl is None:
                    wsl = load_slab(wq, s * 512, 512, f"wq{s}")
                if s + 2 < 4 and (s + 2) not in wsl_pre:
                    wsl_pre[s + 2] = load_slab(wq, (s + 2) * 512, 512,
                                               f"wq{s + 2}")
                for tt in range(NQT):
                    ps = psS.tile([P, 512], f32, tag="S", name="psq")
                    for kt in range(NCT):
                        nc.tensor.matmul(
                            ps,
                            xoT[kt // 4][:, kt % 4, tt * P:(tt + 1) * P],
                            wsl[:, kt, :],
                            start=(kt == 0),
                            stop=(kt == NCT - 1),
                        )
                    cos4, sin4 = cos_tiles(coso, sino, tt * P, 4, f"q{s}{tt}")
                    qhat = qhpool.tile([P, 4, HD], bf16, tag="qhat", name="qhat")
                    rope_rms(ps.rearrange("p (h d) -> p h d", h=4),
                             cos4, sin4, qhat, eps_q, 1.0 / HD, 4)
                    drain_pending(1)
                    pending.append(
                        lambda qhat=qhat, s=s, tt=tt: pack_transpose(
                            qhat,
                            qT[:, 4 * s:4 * s + 4, (3 - tt) * P:(4 - tt) * P],
                            4,
                        )
                    )
            drain_pending()

            # ---------------- phase 2: attention (scores-transposed) -------
            # q-slot columns are stored high-slot-first: the still-valid slots
            # for key tile kt are columns [0, n) with n = 512 - 128*(kt//4).
            # prefetch the first two wo slabs; their DMAs run under phase 2
            w3s = {0: load_slab(wo, 0, 512, "wo0"),
                   1: load_slab(wo, 512, 512, "wo1")}

            tail_state = []  # (yt_psum, den_psum, h)

            def emit_tail():
                if not tail_state:
                    return
                yt, den, h = tail_state.pop(0)
                rinv = smallf.tile([P, QROWS], f32, tag="rq", name="rqinv")
                nc.vector.reciprocal_approx_fast(rinv, den)
                nc.vector.tensor_tensor(yT[:, h, :], yt, rinv, op=OP.mult)

            for h in range(NH):
                kvh = h // (NH // NKV)
                yt = psY.tile([P, QROWS], f32, tag="Y", name="yt")
                den = psD.tile([P, QROWS], f32, tag="D", name="den")
                npv = [0]    # PV matmuls issued (start on 1st, stop on 16th)
                ndone = [0]  # completed presum groups
                accs = {}    # group -> acc AP (first slice of the group)
                gleft = {k: len(v) for k, v in GROUPS.items()}

                # pipeline: fill psum pack-tile -> exp -> per-slice mask,
                # PV matmul, gpsimd presum add; den matmul per finished group.
                work = []  # (pt_tile, slices) awaiting PV/den

                def flush_work(keep, yt=yt, den=den, kvh=kvh,
                               npv=npv, ndone=ndone, accs=accs, gleft=gleft):
                    while len(work) > keep:
                        ptt, slices = work.pop(0)
                        for (off, kt, n) in slices:
                            sl = ptt[:, off:off + n]
                            # causal 0/1 mask on the last 128 cols (own chunk)
                            nc.vector.tensor_tensor(
                                sl[:, n - P:n], sl[:, n - P:n],
                                qmask[:, kt // 4, kt % 4, :], op=OP.mult,
                            )
                            nc.tensor.matmul(
                                yt[:, 0:n], vA[:, kt, kvh, :], sl,
                                start=(npv[0] == 0), stop=(npv[0] == NT - 1),
                                skip_group_check=True,
                            )
                            npv[0] += 1
                            grp = kt // 4
                            if grp not in accs:
                                accs[grp] = sl
                            else:
                                nc.gpsimd.tensor_tensor(
                                    accs[grp], accs[grp], sl, op=OP.add)
                            gleft[grp] -= 1
                            if gleft[grp] == 0:
                                n_g = 512 - 128 * grp
                                nc.tensor.matmul(
                                    den[:, 0:n_g], ones128, accs[grp],
                                    start=(ndone[0] == 0), stop=(ndone[0] == 3),
                                    skip_group_check=True,
                                )
                                ndone[0] += 1

                for (ti, entries) in PACK:
                    W = sum(e[2] for e in entries)
                    S = psS.tile([P, 1024], f32, tag="S", name=f"Sp{ti}")
                    for (off, kt, n) in entries:
                        nc.tensor.matmul(
                            S[:, off:off + n],
                            kT[:, kvh, kt * P:(kt + 1) * P],
                            qT[:, h, 0:n],
                            start=True, stop=True,
                            skip_group_check=True,
                        )
                    if ti == 0 and tail_state:
                        emit_tail()
                    ptt = ptpool.tile([P, 1024], bf16, tag="pt", name="pt")
                    # attn scale already folded into k's rms normalization
                    nc.scalar.activation(ptt[:, 0:W], S[:, 0:W], AF.Exp, scale=1.0)
                    work.append((ptt, entries))
                    flush_work(1)
                flush_work(0)
                tail_state.append((yt, den, h))
            emit_tail()

            # ---------------- phase 3: output projection ----------------
            for s3 in range(4):
                w3 = w3s.pop(s3)
                if s3 + 2 < 4:
                    w3s[s3 + 2] = load_slab(wo, (s3 + 2) * 512, 512,
                                            f"wo{s3 + 2}")
                for qt in range(4):
                    ps = psS.tile([P, 512], f32, tag="S", name="ps3")
                    for ct in range(NCT):
                        nc.tensor.matmul(
                            ps,
                            yT[:, ct, (3 - qt) * P:(4 - qt) * P],
                            w3[:, ct, :],
                            start=(ct == 0),
                            stop=(ct == NCT - 1),
                        )
                    ot = outpool.tile([P, 512], bf16, tag="ot", name="ot")
                    nc.vector.tensor_copy(ot, ps)
                    nc.sync.dma_start(
                        out=yo[qt * P:(qt + 1) * P, s3 * 512:(s3 + 1) * 512],
                        in_=ot,
                    )

    nc.compile()
    return nc


def _get_nc():
    if "nc" not in _CACHE:
        _CACHE["nc"] = _build()
    return _CACHE["nc"]


def _in_maps(x, cosr, sinr, wq, wk, wv, wo):
    xTb = [np.ascontiguousarray(x[b].T).astype(BF) for b in range(B)]
    wqb = np.ascontiguousarray(wq.astype(BF))
    wob = np.ascontiguousarray(wo.astype(BF))
    maps = []
    for core in range(8):
        b, g = core // 4, core % 4
        rows = _rows(g)
        maps.append({
            "xfT": xTb[b],
            "xoTd": np.ascontiguousarray(x[b][rows].T.astype(BF)),
            "cosf": cosr,
            "sinf": sinr,
            "coso": np.ascontiguousarray(cosr[rows]),
            "sino": np.ascontiguousarray(sinr[rows]),
            "wq": wqb,
            "wkvh": np.ascontiguousarray(np.concatenate(
                [wk[:, g * HD:(g + 1) * HD], wv[:, g * HD:(g + 1) * HD]],
                axis=1).astype(BF)),
            "wo": wob,
            "qm": _qmask_t(g),
        })
    return maps


def kernel(x, cos, sin, wq, wk, wv, wo):
    from concourse.bass_utils import run_bass_kernel_spmd

    x = np.ascontiguousarray(np.asarray(x, np.float32))
    cosr = np.ascontiguousarray(np.asarray(cos, np.float32).reshape(T, HD // 2))
    sinr = np.ascontiguousarray(np.asarray(sin, np.float32).reshape(T, HD // 2))
    wq = np.ascontiguousarray(np.asarray(wq, np.float32))
    wk = np.ascontiguousarray(np.asarray(wk, np.float32))
    wv = np.ascontiguousarray(np.asarray(wv, np.float32))
    wo = np.ascontiguousarray(np.asarray(wo, np.float32))

    nc = _get_nc()
    maps = _in_maps(x, cosr, sinr, wq, wk, wv, wo)
    _CACHE["in_maps"] = maps
    res = run_bass_kernel_spmd(nc, maps, list(range(8)))
    y = np.empty((B, T, C), np.float32)
    for core in range(8):
        b, g = core // 4, core % 4
        y[b][_rows(g)] = res.results[core]["yo"].astype(np.float32)
    return y


# revision 16
# speedup vs baseline: 1.3562x; 1.3562x over previous
"""Causal self-attention (GQA, rope, qk-rmsnorm) Trainium2 kernel, 8 NeuronCores.

Sharding: core = (b, g), b = core // 4 (batch), g = core % 4.
Row-sharded Q/attention/output (query row-chunks {g, 4+g, 8+g, 12+g} per core)
with HEAD-SHARDED K/V projection: each core computes only kv-head g's K and V
over all T tokens, then the per-batch 4-core groups AllGather K/V.

v2 scheduling changes vs the 408us baseline:
- attention exp runs on the ACT engine in 5 packed multi-bank psum tiles per
  head (instead of 16 narrow ones) cutting per-instruction bubbles ~3x
- softmax-denominator presum adds moved from the (busy) vector engine to the
  (idle) gpsimd engine
- x slabs double-buffered; initial DMAs ordered so the first matmul starts
  ~4us in (plus PE warmup transposes to ramp the p-state clock early)
- wq/xoTd prefetched during the K/V phase; output stored bf16 (host casts)

Slot c (c = 0..3) covers query chunk 4c+g with keys [0, 512*(c+1)); causal
masking inside the last 512 keys comes from a host-provided 0/1 multiplicative
mask applied to exp(S).
"""

import sys

if "/opt/trn_rl_repo" not in sys.path:
    sys.path.insert(0, "/opt/trn_rl_repo")

import ml_dtypes
import numpy as np

BF = ml_dtypes.bfloat16

B, T, C = 2, 2048, 2048
NH, NKV = 16, 4
HD = C // NH  # 128
P = 128
NT = T // P            # 16 token tiles per batch
NCT = C // P           # 16 contraction tiles
QROWS = 512            # own query rows per core
NQT = QROWS // P       # 4 own token tiles
EPS = float(np.finfo(np.float32).eps)

# attention score-tile packing: 5 psum tiles of [128, 1024] per head.
# each entry: (tile_idx, col_offset, kt, n) where n = valid query cols.
# tile A: kt0,kt1 (n=512); B: kt2,kt3; C: kt4(384),kt12(128),kt5,kt13;
# D: kt6,kt14,kt7,kt15; E: kt8..kt11 (n=256).
PACK = [
    (0, [(0, 0, 512), (512, 1, 512)]),
    (1, [(0, 2, 512), (512, 3, 512)]),
    (2, [(0, 4, 384), (384, 12, 128), (512, 5, 384), (896, 13, 128)]),
    (3, [(0, 6, 384), (384, 14, 128), (512, 7, 384), (896, 15, 128)]),
    (4, [(0, 8, 256), (256, 9, 256), (512, 10, 256), (768, 11, 256)]),
]
# presum groups for the denominator: 4 kt of equal width each; the acc is the
# group's first slice (in issue order).
GROUPS = {0: [0, 1, 2, 3], 1: [4, 5, 6, 7], 2: [8, 9, 10, 11], 3: [12, 13, 14, 15]}

_CACHE = {}


def _chunks(g):
    return [g, 4 + g, 8 + g, 12 + g]


def _rows(g):
    return np.concatenate([np.arange(ch * P, (ch + 1) * P) for ch in _chunks(g)])


def _qbias_t(g):
    """Additive causal bias, transposed layout: (slot c, sub s, k i, q j).

    For slot c the score tile is S^T[k, q] with k in [0, 512*(c+1)) and q the
    128 rows of chunk 4c+g. Only keys in the last 512 of the slot can be
    invalid; bias[c, s, i, j] = 0 if key (512*c + s*128 + i) <= query
    (128*(4c+g) + j) else -30. Accumulated into the S psum by a PE matmul
    with an identity stationary, so exp(S - 30) ~ 1e-13 kills masked keys
    without any vector-engine work.
    """
    m = np.zeros((4, 4, P, P), np.float32)
    for c in range(4):
        k0 = 512 * c
        r0 = (4 * c + g) * P
        k = k0 + np.arange(512)[:, None]          # (512, 1)
        q = r0 + np.arange(P)[None, :]            # (1, 128)
        m[c] = np.where(k <= q, 0.0, -30.0).reshape(4, P, P)
    return m.astype(BF)


def _build():
    import concourse.bacc as bacc
    import concourse.bass as bass
    import concourse.mybir as mybir
    import concourse.tile as tile
    from concourse.masks import make_identity

    f32 = mybir.dt.float32
    bf16 = mybir.dt.bfloat16
    AF = mybir.ActivationFunctionType
    OP = mybir.AluOpType
    AX = mybir.AxisListType

    nc = bacc.Bacc("TRN2", target_bir_lowering=False, debug=False, num_devices=8)

    xfT = nc.dram_tensor("xfT", [C, T], bf16, kind="ExternalInput").ap()
    xoTd = nc.dram_tensor("xoTd", [C, QROWS], bf16, kind="ExternalInput").ap()
    cosf = nc.dram_tensor("cosf", [T, HD // 2], f32, kind="ExternalInput").ap()
    sinf = nc.dram_tensor("sinf", [T, HD // 2], f32, kind="ExternalInput").ap()
    coso = nc.dram_tensor("coso", [QROWS, HD // 2], f32, kind="ExternalInput").ap()
    sino = nc.dram_tensor("sino", [QROWS, HD // 2], f32, kind="ExternalInput").ap()
    wq = nc.dram_tensor("wq", [C, C], bf16, kind="ExternalInput").ap()
    wkvh = nc.dram_tensor("wkvh", [C, 2 * HD], bf16, kind="ExternalInput").ap()
    wo = nc.dram_tensor("wo", [C, C], bf16, kind="ExternalInput").ap()
    qm = nc.dram_tensor("qm", [4, 4, P, P], bf16, kind="ExternalInput").ap()
    yo = nc.dram_tensor("yo", [QROWS, C], bf16, kind="ExternalOutput").ap()

    def bcast4(ap2d):
        # [128, 64] -> [128, 4, 64] with middle step 0 (replicate across heads)
        return bass.AP(
            tensor=ap2d.tensor,
            offset=ap2d.offset,
            ap=[ap2d.ap[0], [0, 4], ap2d.ap[1]],
        )

    with tile.TileContext(nc) as tc:
        with (
            tc.tile_pool(name="singles", bufs=1) as singles,
            tc.tile_pool(name="big", bufs=1) as bigpool,
            tc.tile_pool(name="xq", bufs=2) as xqpool,
            tc.tile_pool(name="slab", bufs=3) as slabpool,
            tc.tile_pool(name="cs", bufs=2) as cspool,
            tc.tile_pool(name="epi", bufs=1) as epipool,
            tc.tile_pool(name="qh", bufs=2) as qhpool,
            tc.tile_pool(name="pt", bufs=4) as ptpool,
            tc.tile_pool(name="smallf", bufs=2) as smallf,
            tc.tile_pool(name="stg", bufs=3) as stgpool,
            tc.tile_pool(name="outs", bufs=2) as outpool,
            tc.tile_pool(name="dram", bufs=1, space="DRAM") as drampool,
            tc.tile_pool(name="psS", bufs=2, space="PSUM") as psS,
            tc.tile_pool(name="psY", bufs=2, space="PSUM") as psY,
            tc.tile_pool(name="psD", bufs=2, space="PSUM") as psD,
        ):
            # ---- first: the loads the first matmuls depend on ----
            def load_slab(w_ap, col0, cols, name):
                """A [C, cols<=512] slice of a weight as [128, 16, cols] bf16.

                All weight slabs (wkv, wq, wo) share one 3-slot pool; the
                phases are disjoint so slots time-share SBUF."""
                wsl = slabpool.tile([P, NCT, 512], bf16, tag="slab", name=name)
                for gr in range(4):
                    nc.sync.dma_start(
                        out=wsl[:, 4 * gr:4 * gr + 4, 0:cols],
                        in_=w_ap[:, col0:col0 + cols].rearrange(
                            "(a p) n -> p a n", p=P
                        )[:, 4 * gr:4 * gr + 4, :],
                    )
                return wsl

            wslkv = load_slab(wkvh, 0, 256, "wkv")

            def load_xe(e):
                """One x eighth ([C, 256]) as 4 tiles of [P, 4, 256]
                (double-buffered)."""
                xh = [xqpool.tile([P, 4, 256], bf16, tag=f"xT{gr}",
                                  name=f"xfT{e}{gr}")
                      for gr in range(4)]
                for gr in range(4):
                    nc.sync.dma_start(
                        out=xh[gr],
                        in_=xfT[gr * 512:(gr + 1) * 512,
                                e * 256:(e + 1) * 256].rearrange(
                            "(a p) n -> p a n", p=P),
                    )
                return xh

            xh_next = load_xe(0)

            ident = singles.tile([P, P], bf16)
            make_identity(nc, ident)

            # PE warmup: dummy transposes ramp the tensor-engine p-state while
            # the first x/w DMAs stream in (~4us of low/mid-clock work).
            warm = psY.tile([P, P], bf16, tag="Y", name="warm")
            for _ in range(24):
                nc.tensor.transpose(warm, ident, ident)

            ones128 = singles.tile([P, P], bf16)
            nc.vector.memset(ones128, 1.0)
            eps_q = singles.tile([P, 1], f32)
            nc.vector.memset(eps_q, EPS)
            eps_k = singles.tile([P, 1], f32)
            nc.vector.memset(eps_k, HD * EPS)

            # persistent big SBUF tensors
            qT = bigpool.tile([P, NH, QROWS], bf16, tag="qT")      # [d, h, q]
            kT = bigpool.tile([P, NKV, T], bf16, tag="kT")         # [d, kvh, k]
            vA = bigpool.tile([P, NT, NKV, HD], bf16, tag="vA")    # [ktok, tt, kvh, d]
            yT = bigpool.tile([P, NCT, QROWS], bf16, tag="yT")     # [d, ct, q]
            qmask = singles.tile([P, 4, 4, P], bf16)               # [ki, c, sub, q]
            nc.sync.dma_start(out=qmask, in_=qm.rearrange("c s i j -> i c s j"))

            # DRAM bounce buffers for the K/V AllGather (two halves so the
            # first collective starts while the second half is computed)
            sendh = [drampool.tile([P, 1024 * (1 if i == 0 else 3)], bf16,
                                   tag=f"send{i}", name=f"sendh{i}")
                     for i in range(2)]
            recvh = [drampool.tile([4, P, 1024 * (1 if i == 0 else 3)], bf16,
                                   tag=f"recv{i}", name=f"recvh{i}")
                     for i in range(2)]

            # ---------------- helpers ----------------
            pending = []  # delayed PE transpose packs (2-deep pipeline)

            def drain_pending(keep=0):
                while len(pending) > keep:
                    pending.pop(0)()

            def rope_rms(v3, cosn, sinn, out_bf, eps_ap, sqrt_scale, nh):
                """v3: [128, nh, 128] psum f32 view. Writes normalized bf16
                rope output to out_bf [128, nh, 128]."""
                ro = epipool.tile([P, 4, HD], f32, tag="ro", name="ro")[:, 0:nh, :]
                cs = epipool.tile([P, 4, HD], f32, tag="cs", name="cs")[:, 0:nh, :]
                sn = epipool.tile([P, 4, HD], f32, tag="sn", name="sn")[:, 0:nh, :]
                nc.vector.tensor_tensor(cs[:, :, 0:64], v3[:, :, 0:64], cosn, op=OP.mult)
                nc.vector.tensor_tensor(cs[:, :, 64:128], v3[:, :, 64:128], cosn, op=OP.mult)
                nc.vector.tensor_tensor(sn[:, :, 0:64], v3[:, :, 0:64], sinn, op=OP.mult)
                nc.vector.tensor_tensor(sn[:, :, 64:128], v3[:, :, 64:128], sinn, op=OP.mult)
                nc.vector.tensor_tensor(ro[:, :, 0:64], cs[:, :, 0:64], sn[:, :, 64:128], op=OP.add)
                nc.vector.tensor_sub(ro[:, :, 64:128], cs[:, :, 64:128], sn[:, :, 0:64])
                ss = smallf.tile([P, 4], f32, tag="ss", name="ss")[:, 0:nh]
                sq = epipool.tile([P, 4, HD], f32, tag="cs", name="sq")[:, 0:nh, :]
                nc.vector.tensor_tensor(sq, ro, ro, op=OP.mult)
                nc.vector.reduce_sum(ss, sq, axis=AX.X)
                rms = smallf.tile([P, 4], f32, tag="rms", name="rms")[:, 0:nh]
                nc.scalar.activation(rms, ss, AF.Sqrt, bias=eps_ap, scale=sqrt_scale)
                rinv = smallf.tile([P, 4], f32, tag="rms", name="rinv")[:, 0:nh]
                nc.vector.reciprocal_approx_fast(rinv, rms)
                for hh in range(nh):
                    nc.vector.tensor_scalar_mul(
                        out_bf[:, hh, :], ro[:, hh, :], rinv[:, hh:hh + 1]
                    )

            def pack_transpose(src_bf, dst3, nh):
                """src_bf [128, nh, 128] bf16 -> nh PE transposes -> one copy
                to dst3 ([128, nh, 128] view)."""
                ptr = psY.tile([P, 512], bf16, tag="Y", name="ptrq")
                for hh in range(nh):
                    nc.tensor.transpose(
                        ptr[:, hh * P:(hh + 1) * P], src_bf[:, hh, :], ident
                    )
                nc.vector.tensor_copy(
                    dst3, ptr[:, 0:nh * P].rearrange("p (s n) -> p s n", s=nh)
                )

            def cos_tiles(cap, sap, t0, nh, name):
                cosn = cspool.tile([P, 4, 64], f32, tag="cs4", name=f"c{name}")[:, 0:nh, :]
                sinn = cspool.tile([P, 4, 64], f32, tag="sn4", name=f"s{name}")[:, 0:nh, :]
                if nh == 4:
                    nc.scalar.dma_start(out=cosn, in_=bcast4(cap[t0:t0 + P, :]))
                    nc.scalar.dma_start(out=sinn, in_=bcast4(sap[t0:t0 + P, :]))
                else:
                    nc.scalar.dma_start(
                        out=cosn,
                        in_=cap[t0:t0 + P, :].rearrange("p (h d) -> p h d", h=1))
                    nc.scalar.dma_start(
                        out=sinn,
                        in_=sap[t0:t0 + P, :].rearrange("p (h d) -> p h d", h=1))
                return cosn, sinn

            # ---------------- phase 0: local kv-head K/V projection --------
            def k_epilogue(kstage, quarter):
                # one 4-wide rope+rms+pack for the whole quarter
                cos4q = cspool.tile([P, 4, 64], f32, tag="cs4", name=f"cq{quarter}")
                sin4q = cspool.tile([P, 4, 64], f32, tag="sn4", name=f"sq{quarter}")
                nc.scalar.dma_start(
                    out=cos4q,
                    in_=cosf[quarter * 512:(quarter + 1) * 512, :].rearrange(
                        "(a p) d -> p a d", p=P))
                nc.scalar.dma_start(
                    out=sin4q,
                    in_=sinf[quarter * 512:(quarter + 1) * 512, :].rearrange(
                        "(a p) d -> p a d", p=P))
                khat4 = qhpool.tile([P, 4, HD], bf16, tag="khat", name=f"kh{quarter}")
                rope_rms(kstage, cos4q, sin4q, khat4, eps_k, 1.0, 4)
                kst4 = stgpool.tile([P, 4, HD], bf16, tag="kst", name=f"kst{quarter}")
                pack_transpose(khat4, kst4, 4)
                if quarter == 0:
                    nc.sync.dma_start(out=sendh[0][:, 0:512],
                                      in_=kst4.rearrange("p a d -> p (a d)"))
                else:
                    q1 = quarter - 1
                    nc.sync.dma_start(out=sendh[1][:, q1 * 512:(q1 + 1) * 512],
                                      in_=kst4.rearrange("p a d -> p (a d)"))

            def kv_allgather(i):
                nc.gpsimd.collective_compute(
                    "AllGather",
                    mybir.AluOpType.bypass,
                    replica_groups=[[0, 1, 2, 3], [4, 5, 6, 7]],
                    ins=[sendh[i].opt()],
                    outs=[recvh[i].opt()],
                )
                kw = 512 if i == 0 else 1536
                t0, t1 = (0, 512) if i == 0 else (512, 2048)
                nc.gpsimd.dma_start(
                    out=kT[:, :, t0:t1],
                    in_=recvh[i][:, :, 0:kw].rearrange("r p n -> p r n"),
                )
                for r in range(4):
                    nc.gpsimd.dma_start(
                        out=vA[:, t0 // P:t1 // P, r, :],
                        in_=recvh[i][r, :, kw:2 * kw].rearrange(
                            "p (tt d) -> p tt d", tt=(t1 - t0) // P),
                    )

            # xoT for the Q projection: loaded early, spread across phase 0
            xoT = [
                bigpool.tile([P, 4, QROWS], bf16, tag=f"xoT{gr}", name=f"xoT{gr}")
                for gr in range(4)
            ]

            def load_xo(tt):
                for gr in range(4):
                    nc.sync.dma_start(
                        out=xoT[gr][:, :, tt * P:(tt + 1) * P],
                        in_=xoTd[gr * 512:(gr + 1) * 512,
                                 tt * P:(tt + 1) * P].rearrange(
                            "(a p) n -> p a n", p=P),
                    )

            wsl_pre = {}
            for quarter in range(4):
                kstage = stgpool.tile([P, 4, HD], f32, tag="kstage",
                                      name=f"kstage{quarter}")
                vstage = stgpool.tile([P, 4, HD], bf16, tag="vstage",
                                      name=f"vstage{quarter}")
                for half in range(2):
                    e = 2 * quarter + half
                    xhT = xh_next
                    if e < 7:
                        xh_next = load_xe(e + 1)
                    # spread the phase-1 prefetches across phase-0 order
                    if e == 0:
                        load_xo(0)
                    elif e == 2:
                        load_xo(1)
                    elif e == 3:
                        load_xo(2)
                        wsl_pre[0] = load_slab(wq, 0, 512, "wq0")
                    elif e == 5:
                        load_xo(3)
                        wsl_pre[1] = load_slab(wq, 512, 512, "wq1")
                    for tl in range(2 * half, 2 * half + 2):
                        tl2 = tl - 2 * half
                        pskv = psS.tile([P, 256], f32, tag="S", name="pskv")
                        for ct in range(NCT):
                            nc.tensor.matmul(
                                pskv,
                                xhT[ct // 4][:, ct % 4, tl2 * P:(tl2 + 1) * P],
                                wslkv[:, ct, 0:256],
                                start=(ct == 0),
                                stop=(ct == NCT - 1),
                            )
                        nc.scalar.copy(kstage[:, tl, :], pskv[:, 0:HD])
                        nc.scalar.copy(vstage[:, tl, :], pskv[:, HD:2 * HD])
                vo0 = 512 if quarter == 0 else 1536 + (quarter - 1) * 512
                nc.sync.dma_start(
                    out=sendh[min(quarter, 1)][:, vo0:vo0 + 512],
                    in_=vstage.rearrange("p a d -> p (a d)"))
                if quarter == 0:
                    k_epilogue(kstage, 0)
                    kv_allgather(0)
                else:
                    drain_pending(1)
                    pending.append(
                        lambda kstage=kstage, quarter=quarter: k_epilogue(
                            kstage, quarter))
            drain_pending()
            kv_allgather(1)

            # ---------------- phase 1: Q projection ----------------
            for s in range(4):
                wsl = wsl_pre.pop(s, None)
                if wsl is None:
                    wsl = load_slab(wq, s * 512, 512, f"wq{s}")
                if s + 2 < 4 and (s + 2) not in wsl_pre:
                    wsl_pre[s + 2] = load_slab(wq, (s + 2) * 512, 512,
                                               f"wq{s + 2}")
                for tt in range(NQT):
                    ps = psS.tile([P, 512], f32, tag="S", name="psq")
                    for kt in range(NCT):
                        nc.tensor.matmul(
                            ps,
                            xoT[kt // 4][:, kt % 4, tt * P:(tt + 1) * P],
                            wsl[:, kt, :],
                            start=(kt == 0),
                            stop=(kt == NCT - 1),
                        )
                    cos4, sin4 = cos_tiles(coso, sino, tt * P, 4, f"q{s}{tt}")
                    qhat = qhpool.tile([P, 4, HD], bf16, tag="qhat", name="qhat")
                    rope_rms(ps.rearrange("p (h d) -> p h d", h=4),
                             cos4, sin4, qhat, eps_q, 1.0 / HD, 4)
                    drain_pending(1)
                    pending.append(
                        lambda qhat=qhat, s=s, tt=tt: pack_transpose(
                            qhat,
                            qT[:, 4 * s:4 * s + 4, (3 - tt) * P:(4 - tt) * P],
                            4,
                        )
                    )
            drain_pending()

            # ---------------- phase 2: attention (scores-transposed) -------
            # q-slot columns are stored high-slot-first: the still-valid slots
            # for key tile kt are columns [0, n) with n = 512 - 128*(kt//4).
            # prefetch the first two wo slabs; their DMAs run under phase 2
            w3s = {0: load_slab(wo, 0, 512, "wo0"),
                   1: load_slab(wo, 512, 512, "wo1")}

            tail_state = []  # (yt_psum, den_psum, h)

            def emit_tail():
                if not tail_state:
                    return
                yt, den, h = tail_state.pop(0)
                rinv = smallf.tile([P, QROWS], f32, tag="rq", name="rqinv")
                nc.vector.reciprocal_approx_fast(rinv, den)
                nc.vector.tensor_tensor(yT[:, h, :], yt, rinv, op=OP.mult)

            for h in range(NH):
                kvh = h // (NH // NKV)
                yt = psY.tile([P, QROWS], f32, tag="Y", name="yt")
                den = psD.tile([P, QROWS], f32, tag="D", name="den")
                npv = [0]    # PV matmuls issued (start on 1st, stop on 16th)
                ndone = [0]  # completed presum groups
                accs = {}    # group -> acc AP (first slice of the group)
                gleft = {k: len(v) for k, v in GROUPS.items()}

                # pipeline: fill psum pack-tile -> exp -> per-slice mask,
                # PV matmul, gpsimd presum add; den matmul per finished group.
                work = []  # (pt_tile, slices) awaiting PV/den

                def flush_work(keep, yt=yt, den=den, kvh=kvh,
                               npv=npv, ndone=ndone, accs=accs, gleft=gleft):
                    while len(work) > keep:
                        ptt, slices = work.pop(0)
                        for (off, kt, n) in slices:
                            sl = ptt[:, off:off + n]
                            nc.tensor.matmul(
                                yt[:, 0:n], vA[:, kt, kvh, :], sl,
                                start=(npv[0] == 0), stop=(npv[0] == NT - 1),
                                skip_group_check=True,
                            )
                            npv[0] += 1
                            grp = kt // 4
                            if grp not in accs:
                                accs[grp] = sl
                            else:
                                nc.vector.tensor_tensor(
                                    accs[grp], accs[grp], sl, op=OP.add)
                            gleft[grp] -= 1
                            if gleft[grp] == 0:
                                n_g = 512 - 128 * grp
                                nc.tensor.matmul(
                                    den[:, 0:n_g], ones128, accs[grp],
                                    start=(ndone[0] == 0), stop=(ndone[0] == 3),
                                    skip_group_check=True,
                                )
                                ndone[0] += 1

                for (ti, entries) in PACK:
                    W = sum(e[2] for e in entries)
                    S = psS.tile([P, 1024], f32, tag="S", name=f"Sp{ti}")
                    for (off, kt, n) in entries:
                        # columns [0, n-128) are fully-valid chunks; the last
                        # 128 (own chunk) get the additive causal bias via a
                        # second accumulating matmul (identity stationary).
                        if n > P:
                            nc.tensor.matmul(
                                S[:, off:off + n - P],
                                kT[:, kvh, kt * P:(kt + 1) * P],
                                qT[:, h, 0:n - P],
                                start=True, stop=True,
                                skip_group_check=True,
                            )
                        nc.tensor.matmul(
                            S[:, off + n - P:off + n],
                            kT[:, kvh, kt * P:(kt + 1) * P],
                            qT[:, h, n - P:n],
                            start=True, stop=False,
                            skip_group_check=True,
                        )
                        nc.tensor.matmul(
                            S[:, off + n - P:off + n],
                            ident,
                            qmask[:, kt // 4, kt % 4, :],
                            start=False, stop=True,
                            skip_group_check=True,
                        )
                    if ti == 0 and tail_state:
                        emit_tail()
                    ptt = ptpool.tile([P, 1024], bf16, tag="pt", name="pt")
                    # attn scale already folded into k's rms normalization
                    nc.scalar.activation(ptt[:, 0:W], S[:, 0:W], AF.Exp, scale=1.0)
                    work.append((ptt, entries))
                    flush_work(1)
                flush_work(0)
                tail_state.append((yt, den, h))
            emit_tail()

            # ---------------- phase 3: output projection ----------------
            for s3 in range(4):
                w3 = w3s.pop(s3)
                if s3 + 2 < 4:
                    w3s[s3 + 2] = load_slab(wo, (s3 + 2) * 512, 512,
                                            f"wo{s3 + 2}")
                for qt in range(4):
                    ps = psS.tile([P, 512], f32, tag="S", name="ps3")
                    for ct in range(NCT):
                        nc.tensor.matmul(
                            ps,
                            yT[:, ct, (3 - qt) * P:(4 - qt) * P],
                            w3[:, ct, :],
                            start=(ct == 0),
                            stop=(ct == NCT - 1),
                        )
                    ot = outpool.tile([P, 512], bf16, tag="ot", name="ot")
                    nc.vector.tensor_copy(ot, ps)
                    nc.sync.dma_start(
                        out=yo[qt * P:(qt + 1) * P, s3 * 512:(s3 + 1) * 512],
                        in_=ot,
                    )

    nc.compile()
    return nc


def _get_nc():
    if "nc" not in _CACHE:
        _CACHE["nc"] = _build()
    return _CACHE["nc"]


def _in_maps(x, cosr, sinr, wq, wk, wv, wo):
    xTb = [np.ascontiguousarray(x[b].T).astype(BF) for b in range(B)]
    wqb = np.ascontiguousarray(wq.astype(BF))
    wob = np.ascontiguousarray(wo.astype(BF))
    maps = []
    for core in range(8):
        b, g = core // 4, core % 4
        rows = _rows(g)
        maps.append({
            "xfT": xTb[b],
            "xoTd": np.ascontiguousarray(x[b][rows].T.astype(BF)),
            "cosf": cosr,
            "sinf": sinr,
            "coso": np.ascontiguousarray(cosr[rows]),
            "sino": np.ascontiguousarray(sinr[rows]),
            "wq": wqb,
            "wkvh": np.ascontiguousarray(np.concatenate(
                [wk[:, g * HD:(g + 1) * HD], wv[:, g * HD:(g + 1) * HD]],
                axis=1).astype(BF)),
            "wo": wob,
            "qm": _qbias_t(g),
        })
    return maps


def kernel(x, cos, sin, wq, wk, wv, wo):
    from concourse.bass_utils import run_bass_kernel_spmd

    x = np.ascontiguousarray(np.asarray(x, np.float32))
    cosr = np.ascontiguousarray(np.asarray(cos, np.float32).reshape(T, HD // 2))
    sinr = np.ascontiguousarray(np.asarray(sin, np.float32).reshape(T, HD // 2))
    wq = np.ascontiguousarray(np.asarray(wq, np.float32))
    wk = np.ascontiguousarray(np.asarray(wk, np.float32))
    wv = np.ascontiguousarray(np.asarray(wv, np.float32))
    wo = np.ascontiguousarray(np.asarray(wo, np.float32))

    nc = _get_nc()
    maps = _in_maps(x, cosr, sinr, wq, wk, wv, wo)
    _CACHE["in_maps"] = maps
    res = run_bass_kernel_spmd(nc, maps, list(range(8)))
    y = np.empty((B, T, C), np.float32)
    for core in range(8):
        b, g = core // 4, core % 4
        y[b][_rows(g)] = res.results[core]["yo"].astype(np.float32)
    return y
